# revision 32
# baseline (speedup 1.0000x reference)
"""Chamfer loss (p3 variant) on 8 Trainium2 NeuronCores.

Computes, for p, q of shape (2, 64, 1024, 4) fp32:
    d2[c,b,n,m] = ||p3[c,b,n] - q3[c,b,m]||^2   (p3 = spatial comps 1:4)
    loss = sum(min_m sqrt(max(d2,0)+1e-12)) + sum(min_n sqrt(...))

Strategy (data-parallel over batch, 8 batches per core):
  - e[n,m] = p3.q3' - 0.5|p3|^2 - 0.5|q3'|^2 = -d2/2, produced directly in
    PSUM by a K=5 matmul over the embedding rows
       lhsT = [x, y, z, -0.5*nrm, 1],  rhs = [x', y', z', 1, -0.5*nrm'].
  - row-min of d2 == -2 * row-max of e. A custom fused DVE op
    (MAXPAIR_REDUCE: out = max(in0,in1), accum = max-reduce) consumes two
    512-wide PSUM/SBUF halves per instruction, so every d2 element crosses
    the DVE at 2 elems/cycle. ScalarE copies one half PSUM->SBUF to enable
    the dual-port read.
  - both passes (p-major row-min and q-major col-min) run as independent
    matmul phases; sqrt (+2 Heron refinements) and the final sum happen on
    a [128, 256] tile of per-chunk minima.
  - default matmul mode "fp16x2": each embedding is split x = hi + lo in
    fp16 (22 combined mantissa bits) and e is accumulated in PSUM from
    3 fp16 matmuls (hi*hi + hi*lo + lo*hi) -- fp32-class accuracy
    (end-to-end rel err ~1e-7) at a fraction of the fp32 matmul cost
    (fp32 runs at 4 cyc/row and its 4-byte weight loads can't use FWL).

Measured on the 8-core axon TRN2 setup: full-input rel err ~2e-6 vs the
fp32 jax reference, ~0.8 ms/iteration sustained (on-device repeat loop;
fp32 matmul variant: 0.91 ms, plain-reduce variant: 0.86 ms).
"""

import os
import sys

sys.path.insert(0, "/opt/trn_rl_repo")

from contextlib import ExitStack

import numpy as np

import concourse.bass as bass
import concourse.tile as tile
from concourse import bacc, mybir

# --------------------------------------------------------------------------
# Custom DVE op: out = max(in0, in1); accum_out = max(s0, max_k out[:, k])
# Registered by appending to concourse.dve_ops.OPS (see
# trainium-docs/custom-instructions/04-custom-dve-api.md).
# --------------------------------------------------------------------------
import concourse.dve_ops as dve_ops
from concourse.dve_ops import DveOp
from concourse.dve_spec import C0, Spec, Src0, Src1, lower as dve_lower, maxx
from concourse.dve_uop import DveOpSpec


def _ref_maxpair_reduce(in0, in1, c0, c1, c2):
    b = np.maximum(in0.astype(np.float32), in1.astype(np.float32))
    P = b.shape[0]
    acc = np.maximum(
        np.broadcast_to(np.asarray(c0, np.float32), (P, 1)),
        b.reshape(P, -1).max(axis=-1, keepdims=True),
    ).astype(np.float32)
    return b, acc


def _register_maxpair():
    spec = Spec(
        body=maxx(Src0, Src1),
        accum=maxx,
        accum_init=C0,
        reference=_ref_maxpair_reduce,
    )
    shas = {}
    for ver in ("v3", "v4"):
        uops = dve_lower(spec, ver=ver)
        shas[ver] = DveOpSpec(
            name="MAXPAIR_REDUCE", opcode=0, uops=uops, rd1_en=True
        ).sha(ver)
    op = DveOp("MAXPAIR_REDUCE", spec, subdim=False, uops_sha=shas)
    if all(o.name != op.name for o in dve_ops.OPS):
        dve_ops.OPS.append(op)
        dve_ops.CUSTOM_DVE_SPECS[op.name] = spec
        dve_ops._SUB_OPCODE_FOR_NAME[op.name] = (
            max(dve_ops._SUB_OPCODE_FOR_NAME.values()) + 1
        )
        assert dve_ops._SUB_OPCODE_FOR_NAME[op.name] < 0x20
    return op


MAXPAIR_REDUCE = _register_maxpair()

# --------------------------------------------------------------------------
# Kernel build
# --------------------------------------------------------------------------
N_CORES = 8
CH = 2  # complex channels
BPC = 8  # batches per core (64 / 8 cores)
N = 1024  # points per set
NCHUNK = N // 128  # partition chunks per batch
F32 = mybir.dt.float32
NEG_SEED = -3.0e38
AX = mybir.AxisListType
ALU = mybir.AluOpType

# matmul operand dtype: "f32" (exact, 4 cyc/row), "bf16" (diagnostic),
# "f32r" (reduced precision, 1 cyc/row), "f32r2" (hi/lo-split f32r:
# 3 accumulated matmuls reconstruct fp32 precision at 3 cyc/row)
MM_DTYPE = os.environ.get("K_MM_DTYPE", "fp16x2")
F32R = mybir.dt.float32r
FP16 = mybir.dt.float16
_MM_DT = {
    "f32": F32,
    "bf16": mybir.dt.bfloat16,
    "f32r": F32R,
    "f32r2": F32R,
    "fp16x2": FP16,
}[MM_DTYPE]
# timing experiments: "" (real), "noprep" (memset embeddings), "nodve"
# (skip custom-DVE reduce), "noact" (skip ACT copies + DVE)
K_EXP = os.environ.get("K_EXP", "")
# reduction mode: "maxpair" (fused dual-port custom DVE op + ACT copy) or
# "plain" (single tensor_reduce over a 2-bank [128,1024] PSUM tile, no ACT)
K_RED = os.environ.get("K_RED", "maxpair")
# K=5 uses 5 of the PE's 128 rows. K_PACK=1 replicates the embeddings at
# base partitions 0/32/64/96 and issues 4 chunk-matmuls concurrently in
# different 32-row strips via tile_position (split mode only).
K_PACK = os.environ.get("K_PACK", "1") == "1"


def _hi_operand(ap):
    """Read a hi-split tile as an operand for fp32 subtraction: f32r shares
    the fp32 bit layout (bitcast); fp16 relies on DVE input dtype
    conversion."""
    return ap.bitcast(F32) if _MM_DT == F32R else ap


def build_kernel(nc, repeat=1):
    p_ap = nc.dram_tensor("p", [CH, BPC, N, 4], F32, kind="ExternalInput").ap()
    q_ap = nc.dram_tensor("q", [CH, BPC, N, 4], F32, kind="ExternalInput").ap()
    out_ap = nc.dram_tensor("out", [1, 1], F32, kind="ExternalOutput").ap()
    inp = [p_ap, q_ap]

    with tile.TileContext(nc) as tc:
        with ExitStack() as ctx:
            dramp = ctx.enter_context(tc.tile_pool(name="dram", bufs=1, space="DRAM"))
            nat = ctx.enter_context(tc.tile_pool(name="nat", bufs=2))
            nrm = ctx.enter_context(tc.tile_pool(name="nrm", bufs=2))
            # (double-buffering the embedding tiles measured slightly worse
            # than single-buffered, and f32r/f32 variants don't fit at bufs=2)
            emb = ctx.enter_context(
                tc.tile_pool(name="emb", bufs=2 if MM_DTYPE == "f32" else 1)
            )
            psp = ctx.enter_context(
                tc.tile_pool(
                    name="psp",
                    bufs=3 if K_RED == "plain" else (7 if K_PACK else 6),
                    space="PSUM",
                )
            )
            in1p = ctx.enter_context(tc.tile_pool(name="in1p", bufs=4))
            scr = ctx.enter_context(tc.tile_pool(name="scr", bufs=3))
            fin = ctx.enter_context(tc.tile_pool(name="fin", bufs=1))
            pssp = ctx.enter_context(tc.tile_pool(name="pssp", bufs=1, space="PSUM"))

            def body(_iv=None):
                split = MM_DTYPE in ("f32r2", "fp16x2")
                st_dt = _MM_DT if split else F32
                # constant rows staged via the same [128, 64] -> flat-n DMA
                # pattern as the norm rows (DVE ops cannot start at
                # partition 3/4, and [1, 8192] SBUF rows cost 32KB/partition)
                ones_f32 = nrm.tile([128, 64], F32, tag="ones_f32")
                nc.vector.memset(ones_f32[:], 1.0)
                if split:
                    # memset can't target f32r; route through a DVE copy
                    # (which is also the "rounding producer" walrus wants)
                    ones_nat = nrm.tile([128, 64], st_dt, tag="ones_nat")
                    nc.vector.tensor_copy(ones_nat[:], ones_f32[:])
                    zf = nrm.tile([128, 64], F32, tag="zf")
                    nc.vector.memset(zf[:], 0.0)
                    zero_nat = nrm.tile([128, 64], st_dt, tag="zero_nat")
                    nc.vector.tensor_copy(zero_nat[:], zf[:])
                else:
                    ones_nat = ones_f32
                    zero_nat = None

                def row_view(st, row):
                    return st[row : row + 1, :].rearrange("o (p u) -> (o p) u", p=128)

                # ---- embedding staging. All DMAs here are contiguous or
                # 256B-run strided. For each (set, ch) build DRAM images of
                #   Lst = [x, y, z, -0.5*nrm, 1]   (lhsT row order)
                #   Rst = [x, y, z, 1, -0.5*nrm]   (rhs row order)
                # in flat-n point order (n = p*64+u matches the natural
                # [128, 64*4] load). In f32r2 mode, hi/lo f32r images are
                # staged instead (value = hi + lo reconstructs fp32).
                lsts, rsts = {}, {}
                for s in range(2):
                    for c in range(CH):
                        pn = nat.tile([128, 256], F32, tag="pn")
                        nc.sync.dma_start(
                            pn[:],
                            inp[s][c].rearrange("b (x u) k -> (b x) (u k)", x=16),
                        )
                        sq = nat.tile([128, 256], F32, tag="sq")
                        nc.scalar.square(sq[:], pn[:])
                        nr = nrm.tile([128, 64], F32, tag="nr")
                        nc.vector.reduce_sum(
                            nr[:],
                            sq[:].rearrange("p (u k) -> p u k", k=4)[:, :, 1:4],
                            axis=AX.X,
                        )
                        nc.vector.tensor_scalar_mul(nr[:], nr[:], -0.5)
                        if split:
                            pnh = nat.tile([128, 256], st_dt, tag="pnh")
                            nc.vector.tensor_copy(pnh[:], pn[:])
                            pnd = nat.tile([128, 256], F32, tag="pnd")
                            nc.vector.tensor_sub(pnd[:], pn[:], _hi_operand(pnh[:]))
                            pnl = nat.tile([128, 256], st_dt, tag="pnl")
                            nc.vector.tensor_copy(pnl[:], pnd[:])
                            nrh = nrm.tile([128, 64], st_dt, tag="nrh")
                            nc.vector.tensor_copy(nrh[:], nr[:])
                            nrd = nrm.tile([128, 64], F32, tag="nrd")
                            nc.vector.tensor_sub(nrd[:], nr[:], _hi_operand(nrh[:]))
                            nrl = nrm.tile([128, 64], st_dt, tag="nrl")
                            nc.vector.tensor_copy(nrl[:], nrd[:])
                            parts = [("h", pnh, nrh, ones_nat), ("l", pnl, nrl, zero_nat)]
                        else:
                            parts = [("", pn, nr, ones_nat)]
                        for sfx, pnx, nrx, onx in parts:
                            cr = nat.tile([128, 192], st_dt, tag="cr")
                            nc.vector.tensor_copy(
                                cr[:].rearrange("p (k u) -> p k u", u=64),
                                pnx[:].rearrange("p (u k) -> p k u", k=4)[:, 1:4, :],
                            )
                            lst = dramp.tile([5, BPC * N], st_dt, tag=f"lst{s}{c}{sfx}")
                            rst = dramp.tile([5, BPC * N], st_dt, tag=f"rst{s}{c}{sfx}")
                            for st, nrow, orow in ((lst, 3, 4), (rst, 4, 3)):
                                nc.sync.dma_start(
                                    st[0:3, :].rearrange("k (p u) -> p k u", p=128),
                                    cr[:].rearrange("p (k u) -> p k u", u=64),
                                )
                                nc.sync.dma_start(row_view(st, nrow), nrx[:])
                                nc.sync.dma_start(row_view(st, orow), onx[:])
                            lsts[(s, c, sfx)] = lst
                            rsts[(s, c, sfx)] = rst

                # ---- accumulator of per-chunk maxima of e = -d2/2
                racc = fin.tile([128, 4 * BPC * NCHUNK], F32, tag="racc")

                col = 0
                for pass_ in range(2):
                    ls, rs = (0, 1) if pass_ == 0 else (1, 0)
                    for c in range(CH):
                        if split and K_PACK:
                            # replicas of the 5 embedding rows at base
                            # partitions 0/32/64/96 for 4-way row-packed MMs
                            Lh = emb.tile([101, BPC * N], st_dt, tag="Lh")
                            Ll = emb.tile([101, BPC * N], st_dt, tag="Ll")
                            Rh = emb.tile([101, BPC * N], st_dt, tag="Rh")
                            Rl = emb.tile([101, BPC * N], st_dt, tag="Rl")
                            for t, src in (
                                (Lh, lsts[(ls, c, "h")]),
                                (Ll, lsts[(ls, c, "l")]),
                                (Rh, rsts[(rs, c, "h")]),
                                (Rl, rsts[(rs, c, "l")]),
                            ):
                                for g in range(4):
                                    nc.sync.dma_start(
                                        t[32 * g : 32 * g + 5, :], src[:]
                                    )
                        elif split:
                            Lh = emb.tile([5, BPC * N], st_dt, tag="Lh")
                            nc.sync.dma_start(Lh[:], lsts[(ls, c, "h")][:])
                            Ll = emb.tile([5, BPC * N], st_dt, tag="Ll")
                            nc.sync.dma_start(Ll[:], lsts[(ls, c, "l")][:])
                            Rh = emb.tile([5, BPC * N], st_dt, tag="Rh")
                            nc.sync.dma_start(Rh[:], rsts[(rs, c, "h")][:])
                            Rl = emb.tile([5, BPC * N], st_dt, tag="Rl")
                            nc.sync.dma_start(Rl[:], rsts[(rs, c, "l")][:])
                        else:
                            L = emb.tile([5, BPC * N], F32, tag="L")
                            R = emb.tile([5, BPC * N], F32, tag="R")
                            if K_EXP == "noprep":
                                nc.vector.memset(L[:], 0.25)
                                nc.vector.memset(R[:], 0.5)
                            else:
                                nc.sync.dma_start(L[:], lsts[(ls, c, "")][:])
                                nc.sync.dma_start(R[:], rsts[(rs, c, "")][:])
                            if MM_DTYPE in ("bf16", "f32r"):
                                Lm = emb.tile([5, BPC * N], _MM_DT, tag="Lm")
                                nc.vector.tensor_copy(Lm[:], L[:])
                                Rm = emb.tile([5, BPC * N], _MM_DT, tag="Rm")
                                nc.vector.tensor_copy(Rm[:], R[:])
                                L, R = Lm, Rm

                        def mm_tile(ps, b, i, j, g=0):
                            lo = b * N + i * 128
                            hi = lo + 128
                            mlo = b * N + j * 512
                            mhi = mlo + 512
                            pbase = 32 * g
                            tp = {"tile_position": (pbase, 0)} if K_PACK else {}
                            if split:
                                nc.tensor.matmul(
                                    ps[:],
                                    Lh[pbase : pbase + 5, lo:hi],
                                    Rh[pbase : pbase + 5, mlo:mhi],
                                    start=True, stop=False, **tp,
                                )
                                nc.tensor.matmul(
                                    ps[:],
                                    Lh[pbase : pbase + 5, lo:hi],
                                    Rl[pbase : pbase + 5, mlo:mhi],
                                    start=False, stop=False, **tp,
                                )
                                nc.tensor.matmul(
                                    ps[:],
                                    Ll[pbase : pbase + 5, lo:hi],
                                    Rh[pbase : pbase + 5, mlo:mhi],
                                    start=False, stop=True, **tp,
                                )
                            else:
                                nc.tensor.matmul(
                                    ps[:], L[:, lo:hi], R[:, mlo:mhi],
                                    start=True, stop=True,
                                )

                        if split and K_PACK:
                            # 4 chunks per quad run concurrently in distinct
                            # 32-row PE strips; consumers drain per chunk.
                            for b in range(BPC):
                                for iq in range(NCHUNK // 4):
                                    pss = {}
                                    for j in range(2):
                                        for g in range(4):
                                            ps = psp.tile([128, 512], F32, tag="ps")
                                            mm_tile(ps, b, iq * 4 + g, j, g)
                                            pss[(j, g)] = ps
                                    for g in range(4):
                                        buf1 = in1p.tile([128, 512], F32, tag="b1")
                                        nc.scalar.copy(buf1[:], pss[(1, g)][:])
                                        sc = scr.tile([128, 512], F32, tag="sc")
                                        nc.vector._custom_dve(
                                            MAXPAIR_REDUCE,
                                            out=sc[:],
                                            in0=pss[(0, g)][:],
                                            in1=buf1[:],
                                            s0=NEG_SEED,
                                            accum_out=racc[:, col : col + 1],
                                        )
                                        col += 1
                            continue

                        for b in range(BPC):
                            for i in range(NCHUNK):
                                if K_RED == "plain":
                                    psw = psp.tile([128, 1024], F32, tag="psw")
                                    mm_tile(psw[:, 0:512], b, i, 0)
                                    mm_tile(psw[:, 512:1024], b, i, 1)
                                    nc.vector.tensor_reduce(
                                        racc[:, col : col + 1],
                                        psw[:],
                                        axis=AX.X,
                                        op=ALU.max,
                                    )
                                    col += 1
                                    continue
                                ps0 = psp.tile([128, 512], F32, tag="ps")
                                mm_tile(ps0, b, i, 0)
                                ps1 = psp.tile([128, 512], F32, tag="ps")
                                mm_tile(ps1, b, i, 1)
                                if K_EXP == "noact":
                                    col += 1
                                    continue
                                buf1 = in1p.tile([128, 512], F32, tag="b1")
                                nc.scalar.copy(buf1[:], ps1[:])
                                if K_EXP == "nodve":
                                    col += 1
                                    continue
                                sc = scr.tile([128, 512], F32, tag="sc")
                                nc.vector._custom_dve(
                                    MAXPAIR_REDUCE,
                                    out=sc[:],
                                    in0=ps0[:],
                                    in1=buf1[:],
                                    s0=NEG_SEED,
                                    accum_out=racc[:, col : col + 1],
                                )
                                col += 1

                # ---- finale: d2min = -2*min(racc,0); dist = sqrt(d2min+1e-12)
                # (2 Heron steps refine ScalarE's spline sqrt); sum everything.
                if K_EXP in ("nodve", "noact"):
                    nc.vector.memset(racc[:], -1.0)
                ncols = col
                u = fin.tile([128, ncols], F32, tag="u")
                nc.vector.tensor_scalar_min(u[:], racc[:], 0.0)
                x = fin.tile([128, ncols], F32, tag="x")
                nc.vector.tensor_scalar(x[:], u[:], -2.0, 1e-12, ALU.mult, ALU.add)
                s0t = fin.tile([128, ncols], F32, tag="s0t")
                nc.scalar.sqrt(s0t[:], x[:])
                st = s0t
                for _ in range(2):
                    r = fin.tile([128, ncols], F32, tag="r")
                    nc.vector.reciprocal(r[:], st[:])
                    t = fin.tile([128, ncols], F32, tag="t")
                    nc.vector.tensor_mul(t[:], x[:], r[:])
                    v = fin.tile([128, ncols], F32, tag="v")
                    nc.vector.tensor_add(v[:], st[:], t[:])
                    s2 = fin.tile([128, ncols], F32, tag="s2")
                    nc.vector.tensor_scalar_mul(s2[:], v[:], 0.5)
                    st = s2
                z = fin.tile([128, 1], F32, tag="z")
                nc.vector.reduce_sum(z[:], st[:], axis=AX.X)
                ones = fin.tile([128, 1], F32, tag="ones")
                nc.vector.memset(ones[:], 1.0)
                pss = pssp.tile([1, 1], F32, tag="pss")
                nc.tensor.matmul(pss[:], z[:], ones[:], start=True, stop=True)
                ob = fin.tile([1, 1], F32, tag="ob")
                nc.scalar.copy(ob[:], pss[:])
                nc.sync.dma_start(out_ap[:], ob[:])

            if repeat == 1:
                body()
            else:
                with tc.For_i(0, repeat, 1) as _i:
                    body(_i)
    return nc


_CACHE = {}


def _get_compiled(repeat=1):
    if repeat not in _CACHE:
        nc = bacc.Bacc(
            "TRN2", target_bir_lowering=False, debug=False, num_devices=N_CORES
        )
        build_kernel(nc, repeat=repeat)
        nc.compile()
        _CACHE[repeat] = nc
    return _CACHE[repeat]


def kernel(p, q):
    """Full-input chamfer loss; shards batch dim over 8 NeuronCores."""
    from concourse.bass_utils import run_bass_kernel_spmd

    p = np.asarray(p, dtype=np.float32)
    q = np.asarray(q, dtype=np.float32)
    assert p.shape == (CH, N_CORES * BPC, N, 4) and q.shape == p.shape

    nc = _get_compiled(repeat=1)
    in_maps = [
        {
            "p": np.ascontiguousarray(p[:, k * BPC : (k + 1) * BPC]),
            "q": np.ascontiguousarray(q[:, k * BPC : (k + 1) * BPC]),
        }
        for k in range(N_CORES)
    ]
    res = run_bass_kernel_spmd(nc, in_maps, list(range(N_CORES)))
    total = np.float32(0.0)
    for k in range(N_CORES):
        total += np.float32(res.results[k]["out"].reshape(()))
    return np.asarray(total, dtype=np.float32).reshape(())


# revision 33
# speedup vs baseline: 1.0729x; 1.0729x over previous
"""Chamfer loss (p3 variant) on 8 Trainium2 NeuronCores.

Computes, for p, q of shape (2, 64, 1024, 4) fp32:
    d2[c,b,n,m] = ||p3[c,b,n] - q3[c,b,m]||^2   (p3 = spatial comps 1:4)
    loss = sum(min_m sqrt(max(d2,0)+1e-12)) + sum(min_n sqrt(...))

Strategy (data-parallel over batch, 8 batches per core):
  - e[n,m] = p3.q3' - 0.5|p3|^2 - 0.5|q3'|^2 = -d2/2, produced directly in
    PSUM by a K=5 matmul over the embedding rows
       lhsT = [x, y, z, -0.5*nrm, 1],  rhs = [x', y', z', 1, -0.5*nrm'].
  - row-min of d2 == -2 * row-max of e. A custom fused DVE op
    (MAXPAIR_REDUCE: out = max(in0,in1), accum = max-reduce) consumes two
    512-wide PSUM/SBUF halves per instruction, so every d2 element crosses
    the DVE at 2 elems/cycle. ScalarE copies one half PSUM->SBUF to enable
    the dual-port read.
  - both passes (p-major row-min and q-major col-min) run as independent
    matmul phases; sqrt (+2 Heron refinements) and the final sum happen on
    a [128, 256] tile of per-chunk minima.
  - default matmul mode "fp16x2": each embedding is split x = hi + lo in
    fp16 (22 combined mantissa bits) and e is accumulated in PSUM from
    3 fp16 matmuls (hi*hi + hi*lo + lo*hi) -- fp32-class accuracy
    (end-to-end rel err ~1e-7) at a fraction of the fp32 matmul cost
    (fp32 runs at 4 cyc/row and its 4-byte weight loads can't use FWL).

Measured on the 8-core axon TRN2 setup: full-input rel err ~2e-6 vs the
fp32 jax reference, ~0.8 ms/iteration sustained (on-device repeat loop;
fp32 matmul variant: 0.91 ms, plain-reduce variant: 0.86 ms).
"""

import os
import sys

sys.path.insert(0, "/opt/trn_rl_repo")

from contextlib import ExitStack

import numpy as np

import concourse.bass as bass
import concourse.tile as tile
from concourse import bacc, mybir

# --------------------------------------------------------------------------
# Custom DVE op: out = max(in0, in1); accum_out = max(s0, max_k out[:, k])
# Registered by appending to concourse.dve_ops.OPS (see
# trainium-docs/custom-instructions/04-custom-dve-api.md).
# --------------------------------------------------------------------------
import concourse.dve_ops as dve_ops
from concourse.dve_ops import DveOp
from concourse.dve_spec import C0, Spec, Src0, Src1, lower as dve_lower, maxx
from concourse.dve_uop import DveOpSpec


def _ref_maxpair_reduce(in0, in1, c0, c1, c2):
    b = np.maximum(in0.astype(np.float32), in1.astype(np.float32))
    P = b.shape[0]
    acc = np.maximum(
        np.broadcast_to(np.asarray(c0, np.float32), (P, 1)),
        b.reshape(P, -1).max(axis=-1, keepdims=True),
    ).astype(np.float32)
    return b, acc


def _register_maxpair():
    spec = Spec(
        body=maxx(Src0, Src1),
        accum=maxx,
        accum_init=C0,
        reference=_ref_maxpair_reduce,
    )
    shas = {}
    for ver in ("v3", "v4"):
        uops = dve_lower(spec, ver=ver)
        shas[ver] = DveOpSpec(
            name="MAXPAIR_REDUCE", opcode=0, uops=uops, rd1_en=True
        ).sha(ver)
    op = DveOp("MAXPAIR_REDUCE", spec, subdim=False, uops_sha=shas)
    if all(o.name != op.name for o in dve_ops.OPS):
        dve_ops.OPS.append(op)
        dve_ops.CUSTOM_DVE_SPECS[op.name] = spec
        dve_ops._SUB_OPCODE_FOR_NAME[op.name] = (
            max(dve_ops._SUB_OPCODE_FOR_NAME.values()) + 1
        )
        assert dve_ops._SUB_OPCODE_FOR_NAME[op.name] < 0x20
    return op


MAXPAIR_REDUCE = _register_maxpair()

# --------------------------------------------------------------------------
# Kernel build
# --------------------------------------------------------------------------
N_CORES = 8
CH = 2  # complex channels
BPC = 8  # batches per core (64 / 8 cores)
N = 1024  # points per set
NCHUNK = N // 128  # partition chunks per batch
F32 = mybir.dt.float32
NEG_SEED = -3.0e38
AX = mybir.AxisListType
ALU = mybir.AluOpType

# matmul operand dtype: "f32" (exact, 4 cyc/row), "bf16" (diagnostic),
# "f32r" (reduced precision, 1 cyc/row), "f32r2" (hi/lo-split f32r:
# 3 accumulated matmuls reconstruct fp32 precision at 3 cyc/row)
MM_DTYPE = os.environ.get("K_MM_DTYPE", "fp16x2")
F32R = mybir.dt.float32r
FP16 = mybir.dt.float16
_MM_DT = {
    "f32": F32,
    "bf16": mybir.dt.bfloat16,
    "f32r": F32R,
    "f32r2": F32R,
    "fp16x2": FP16,
}[MM_DTYPE]
# timing experiments: "" (real), "noprep" (memset embeddings), "nodve"
# (skip custom-DVE reduce), "noact" (skip ACT copies + DVE)
K_EXP = os.environ.get("K_EXP", "")
# reduction mode: "maxpair" (fused dual-port custom DVE op + ACT copy) or
# "plain" (single tensor_reduce over a 2-bank [128,1024] PSUM tile, no ACT)
K_RED = os.environ.get("K_RED", "maxpair")
# K=5 uses 5 of the PE's 128 rows. K_PACK=1 replicates the embeddings at
# base partitions 0/32/64/96 and issues 4 chunk-matmuls concurrently in
# different 32-row strips via tile_position (split mode only).
K_PACK = os.environ.get("K_PACK", "1") == "1"


def _hi_operand(ap):
    """Read a hi-split tile as an operand for fp32 subtraction: f32r shares
    the fp32 bit layout (bitcast); fp16 relies on DVE input dtype
    conversion."""
    return ap.bitcast(F32) if _MM_DT == F32R else ap


def build_kernel(nc, repeat=1):
    p_ap = nc.dram_tensor("p", [CH, BPC, N, 4], F32, kind="ExternalInput").ap()
    q_ap = nc.dram_tensor("q", [CH, BPC, N, 4], F32, kind="ExternalInput").ap()
    out_ap = nc.dram_tensor("out", [1, 1], F32, kind="ExternalOutput").ap()
    inp = [p_ap, q_ap]

    with tile.TileContext(nc) as tc:
        with ExitStack() as ctx:
            dramp = ctx.enter_context(tc.tile_pool(name="dram", bufs=1, space="DRAM"))
            nat = ctx.enter_context(tc.tile_pool(name="nat", bufs=2))
            nrm = ctx.enter_context(tc.tile_pool(name="nrm", bufs=2))
            # (double-buffering the embedding tiles measured slightly worse
            # than single-buffered, and f32r/f32 variants don't fit at bufs=2)
            emb = ctx.enter_context(
                tc.tile_pool(name="emb", bufs=2 if MM_DTYPE == "f32" else 1)
            )
            psp = ctx.enter_context(
                tc.tile_pool(
                    name="psp",
                    bufs=3 if K_RED == "plain" else (8 if K_PACK else 6),
                    space="PSUM",
                )
            )
            in1p = ctx.enter_context(tc.tile_pool(name="in1p", bufs=4))
            scr = ctx.enter_context(tc.tile_pool(name="scr", bufs=3))
            fin = ctx.enter_context(tc.tile_pool(name="fin", bufs=1))

            def body(_iv=None):
                split = MM_DTYPE in ("f32r2", "fp16x2")
                st_dt = _MM_DT if split else F32
                # constant rows staged via the same [128, 64] -> flat-n DMA
                # pattern as the norm rows (DVE ops cannot start at
                # partition 3/4, and [1, 8192] SBUF rows cost 32KB/partition)
                ones_f32 = nrm.tile([128, 64], F32, tag="ones_f32")
                nc.vector.memset(ones_f32[:], 1.0)
                if split:
                    # memset can't target f32r; route through a DVE copy
                    # (which is also the "rounding producer" walrus wants)
                    ones_nat = nrm.tile([128, 64], st_dt, tag="ones_nat")
                    nc.vector.tensor_copy(ones_nat[:], ones_f32[:])
                    zf = nrm.tile([128, 64], F32, tag="zf")
                    nc.vector.memset(zf[:], 0.0)
                    zero_nat = nrm.tile([128, 64], st_dt, tag="zero_nat")
                    nc.vector.tensor_copy(zero_nat[:], zf[:])
                else:
                    ones_nat = ones_f32
                    zero_nat = None

                def row_view(st, row):
                    return st[row : row + 1, :].rearrange("o (p u) -> (o p) u", p=128)

                # ---- embedding staging. All DMAs here are contiguous or
                # 256B-run strided. For each (set, ch) build DRAM images of
                #   Lst = [x, y, z, -0.5*nrm, 1]   (lhsT row order)
                #   Rst = [x, y, z, 1, -0.5*nrm]   (rhs row order)
                # in flat-n point order (n = p*64+u matches the natural
                # [128, 64*4] load). In f32r2 mode, hi/lo f32r images are
                # staged instead (value = hi + lo reconstructs fp32).
                lsts, rsts = {}, {}
                for s in range(2):
                    for c in range(CH):
                        pn = nat.tile([128, 256], F32, tag="pn")
                        nc.sync.dma_start(
                            pn[:],
                            inp[s][c].rearrange("b (x u) k -> (b x) (u k)", x=16),
                        )
                        sq = nat.tile([128, 256], F32, tag="sq")
                        nc.scalar.square(sq[:], pn[:])
                        nr = nrm.tile([128, 64], F32, tag="nr")
                        nc.vector.reduce_sum(
                            nr[:],
                            sq[:].rearrange("p (u k) -> p u k", k=4)[:, :, 1:4],
                            axis=AX.X,
                        )
                        nc.vector.tensor_scalar_mul(nr[:], nr[:], -0.5)
                        if split:
                            pnh = nat.tile([128, 256], st_dt, tag="pnh")
                            nc.vector.tensor_copy(pnh[:], pn[:])
                            pnd = nat.tile([128, 256], F32, tag="pnd")
                            nc.vector.tensor_sub(pnd[:], pn[:], _hi_operand(pnh[:]))
                            pnl = nat.tile([128, 256], st_dt, tag="pnl")
                            nc.vector.tensor_copy(pnl[:], pnd[:])
                            nrh = nrm.tile([128, 64], st_dt, tag="nrh")
                            nc.vector.tensor_copy(nrh[:], nr[:])
                            nrd = nrm.tile([128, 64], F32, tag="nrd")
                            nc.vector.tensor_sub(nrd[:], nr[:], _hi_operand(nrh[:]))
                            nrl = nrm.tile([128, 64], st_dt, tag="nrl")
                            nc.vector.tensor_copy(nrl[:], nrd[:])
                            parts = [("h", pnh, nrh, ones_nat), ("l", pnl, nrl, zero_nat)]
                        else:
                            parts = [("", pn, nr, ones_nat)]
                        for sfx, pnx, nrx, onx in parts:
                            cr = nat.tile([128, 192], st_dt, tag="cr")
                            nc.vector.tensor_copy(
                                cr[:].rearrange("p (k u) -> p k u", u=64),
                                pnx[:].rearrange("p (u k) -> p k u", k=4)[:, 1:4, :],
                            )
                            lst = dramp.tile([5, BPC * N], st_dt, tag=f"lst{s}{c}{sfx}")
                            rst = dramp.tile([5, BPC * N], st_dt, tag=f"rst{s}{c}{sfx}")
                            for st, nrow, orow in ((lst, 3, 4), (rst, 4, 3)):
                                nc.sync.dma_start(
                                    st[0:3, :].rearrange("k (p u) -> p k u", p=128),
                                    cr[:].rearrange("p (k u) -> p k u", u=64),
                                )
                                nc.sync.dma_start(row_view(st, nrow), nrx[:])
                                nc.sync.dma_start(row_view(st, orow), onx[:])
                            lsts[(s, c, sfx)] = lst
                            rsts[(s, c, sfx)] = rst

                # ---- accumulator of per-chunk maxima of e = -d2/2
                racc = fin.tile([128, 4 * BPC * NCHUNK], F32, tag="racc")

                col = 0
                for pass_ in range(2):
                    ls, rs = (0, 1) if pass_ == 0 else (1, 0)
                    for c in range(CH):
                        if split and K_PACK:
                            # replicas of the 5 embedding rows at base
                            # partitions 0/32/64/96 for 4-way row-packed MMs
                            Lh = emb.tile([101, BPC * N], st_dt, tag="Lh")
                            Ll = emb.tile([101, BPC * N], st_dt, tag="Ll")
                            Rh = emb.tile([101, BPC * N], st_dt, tag="Rh")
                            Rl = emb.tile([101, BPC * N], st_dt, tag="Rl")
                            for t, src in (
                                (Lh, lsts[(ls, c, "h")]),
                                (Ll, lsts[(ls, c, "l")]),
                                (Rh, rsts[(rs, c, "h")]),
                                (Rl, rsts[(rs, c, "l")]),
                            ):
                                for g in range(4):
                                    nc.sync.dma_start(
                                        t[32 * g : 32 * g + 5, :], src[:]
                                    )
                        elif split:
                            Lh = emb.tile([5, BPC * N], st_dt, tag="Lh")
                            nc.sync.dma_start(Lh[:], lsts[(ls, c, "h")][:])
                            Ll = emb.tile([5, BPC * N], st_dt, tag="Ll")
                            nc.sync.dma_start(Ll[:], lsts[(ls, c, "l")][:])
                            Rh = emb.tile([5, BPC * N], st_dt, tag="Rh")
                            nc.sync.dma_start(Rh[:], rsts[(rs, c, "h")][:])
                            Rl = emb.tile([5, BPC * N], st_dt, tag="Rl")
                            nc.sync.dma_start(Rl[:], rsts[(rs, c, "l")][:])
                        else:
                            L = emb.tile([5, BPC * N], F32, tag="L")
                            R = emb.tile([5, BPC * N], F32, tag="R")
                            if K_EXP == "noprep":
                                nc.vector.memset(L[:], 0.25)
                                nc.vector.memset(R[:], 0.5)
                            else:
                                nc.sync.dma_start(L[:], lsts[(ls, c, "")][:])
                                nc.sync.dma_start(R[:], rsts[(rs, c, "")][:])
                            if MM_DTYPE in ("bf16", "f32r"):
                                Lm = emb.tile([5, BPC * N], _MM_DT, tag="Lm")
                                nc.vector.tensor_copy(Lm[:], L[:])
                                Rm = emb.tile([5, BPC * N], _MM_DT, tag="Rm")
                                nc.vector.tensor_copy(Rm[:], R[:])
                                L, R = Lm, Rm

                        def mm_tile(ps, b, i, j, g=0):
                            lo = b * N + i * 128
                            hi = lo + 128
                            mlo = b * N + j * 512
                            mhi = mlo + 512
                            pbase = 32 * g
                            tp = {"tile_position": (pbase, 0)} if K_PACK else {}
                            if split:
                                nc.tensor.matmul(
                                    ps[:],
                                    Lh[pbase : pbase + 5, lo:hi],
                                    Rh[pbase : pbase + 5, mlo:mhi],
                                    start=True, stop=False, **tp,
                                )
                                nc.tensor.matmul(
                                    ps[:],
                                    Lh[pbase : pbase + 5, lo:hi],
                                    Rl[pbase : pbase + 5, mlo:mhi],
                                    start=False, stop=False, **tp,
                                )
                                nc.tensor.matmul(
                                    ps[:],
                                    Ll[pbase : pbase + 5, lo:hi],
                                    Rh[pbase : pbase + 5, mlo:mhi],
                                    start=False, stop=True, **tp,
                                )
                            else:
                                nc.tensor.matmul(
                                    ps[:], L[:, lo:hi], R[:, mlo:mhi],
                                    start=True, stop=True,
                                )

                        if split and K_PACK:
                            # 4 chunks per quad run concurrently in distinct
                            # 32-row PE strips; consumers drain per chunk.
                            for b in range(BPC):
                                for iq in range(NCHUNK // 4):
                                    pss = {}
                                    for j in range(2):
                                        for g in range(4):
                                            ps = psp.tile([128, 512], F32, tag="ps")
                                            mm_tile(ps, b, iq * 4 + g, j, g)
                                            pss[(j, g)] = ps
                                    for g in range(4):
                                        buf1 = in1p.tile([128, 512], F32, tag="b1")
                                        nc.scalar.copy(buf1[:], pss[(1, g)][:])
                                        sc = scr.tile([128, 512], F32, tag="sc")
                                        nc.vector._custom_dve(
                                            MAXPAIR_REDUCE,
                                            out=sc[:],
                                            in0=pss[(0, g)][:],
                                            in1=buf1[:],
                                            s0=NEG_SEED,
                                            accum_out=racc[:, col : col + 1],
                                        )
                                        col += 1
                            continue

                        for b in range(BPC):
                            for i in range(NCHUNK):
                                if K_RED == "plain":
                                    psw = psp.tile([128, 1024], F32, tag="psw")
                                    mm_tile(psw[:, 0:512], b, i, 0)
                                    mm_tile(psw[:, 512:1024], b, i, 1)
                                    nc.vector.tensor_reduce(
                                        racc[:, col : col + 1],
                                        psw[:],
                                        axis=AX.X,
                                        op=ALU.max,
                                    )
                                    col += 1
                                    continue
                                ps0 = psp.tile([128, 512], F32, tag="ps")
                                mm_tile(ps0, b, i, 0)
                                ps1 = psp.tile([128, 512], F32, tag="ps")
                                mm_tile(ps1, b, i, 1)
                                if K_EXP == "noact":
                                    col += 1
                                    continue
                                buf1 = in1p.tile([128, 512], F32, tag="b1")
                                nc.scalar.copy(buf1[:], ps1[:])
                                if K_EXP == "nodve":
                                    col += 1
                                    continue
                                sc = scr.tile([128, 512], F32, tag="sc")
                                nc.vector._custom_dve(
                                    MAXPAIR_REDUCE,
                                    out=sc[:],
                                    in0=ps0[:],
                                    in1=buf1[:],
                                    s0=NEG_SEED,
                                    accum_out=racc[:, col : col + 1],
                                )
                                col += 1

                # ---- finale: d2min = -2*min(racc,0); dist = sqrt(d2min+1e-12)
                # (2 Heron steps refine ScalarE's spline sqrt); sum everything.
                if K_EXP in ("nodve", "noact"):
                    nc.vector.memset(racc[:], -1.0)
                ncols = col
                u = fin.tile([128, ncols], F32, tag="u")
                nc.vector.tensor_scalar_min(u[:], racc[:], 0.0)
                x = fin.tile([128, ncols], F32, tag="x")
                nc.vector.tensor_scalar(x[:], u[:], -2.0, 1e-12, ALU.mult, ALU.add)
                s0t = fin.tile([128, ncols], F32, tag="s0t")
                nc.scalar.sqrt(s0t[:], x[:])
                st = s0t
                for _ in range(2):
                    r = fin.tile([128, ncols], F32, tag="r")
                    nc.vector.reciprocal(r[:], st[:])
                    t = fin.tile([128, ncols], F32, tag="t")
                    nc.vector.tensor_mul(t[:], x[:], r[:])
                    v = fin.tile([128, ncols], F32, tag="v")
                    nc.vector.tensor_add(v[:], st[:], t[:])
                    s2 = fin.tile([128, ncols], F32, tag="s2")
                    nc.vector.tensor_scalar_mul(s2[:], v[:], 0.5)
                    st = s2
                z = fin.tile([128, 1], F32, tag="z")
                nc.vector.reduce_sum(z[:], st[:], axis=AX.X)
                ones = fin.tile([128, 1], F32, tag="ones")
                nc.vector.memset(ones[:], 1.0)
                pss = psp.tile([1, 1], F32, tag="ps")
                nc.tensor.matmul(pss[:], z[:], ones[:], start=True, stop=True)
                ob = fin.tile([1, 1], F32, tag="ob")
                nc.scalar.copy(ob[:], pss[:])
                nc.sync.dma_start(out_ap[:], ob[:])

            if repeat == 1:
                body()
            else:
                with tc.For_i(0, repeat, 1) as _i:
                    body(_i)
    return nc


_CACHE = {}


def _get_compiled(repeat=1):
    if repeat not in _CACHE:
        nc = bacc.Bacc(
            "TRN2", target_bir_lowering=False, debug=False, num_devices=N_CORES
        )
        build_kernel(nc, repeat=repeat)
        nc.compile()
        _CACHE[repeat] = nc
    return _CACHE[repeat]


def kernel(p, q):
    """Full-input chamfer loss; shards batch dim over 8 NeuronCores."""
    from concourse.bass_utils import run_bass_kernel_spmd

    p = np.asarray(p, dtype=np.float32)
    q = np.asarray(q, dtype=np.float32)
    assert p.shape == (CH, N_CORES * BPC, N, 4) and q.shape == p.shape

    nc = _get_compiled(repeat=1)
    in_maps = [
        {
            "p": np.ascontiguousarray(p[:, k * BPC : (k + 1) * BPC]),
            "q": np.ascontiguousarray(q[:, k * BPC : (k + 1) * BPC]),
        }
        for k in range(N_CORES)
    ]
    res = run_bass_kernel_spmd(nc, in_maps, list(range(N_CORES)))
    total = np.float32(0.0)
    for k in range(N_CORES):
        total += np.float32(res.results[k]["out"].reshape(()))
    return np.asarray(total, dtype=np.float32).reshape(())
